# revision 23
# baseline (speedup 1.0000x reference)
"""Multi-head attention TRN2 kernel (v5, fp16, cascaded window pipeline).

Sharding: 8 cores = 4 batches x 2 head-groups (Megatron tensor parallel over
the 16 heads: Wq/Wk/Wv column-sharded, Wo row-sharded; partial outputs summed
per batch on the host).

Per-core schedule (batch b, head-group g -> 8 local heads, 4 head-pairs c,
window = (s-tile st, head-pair c), 16 windows):
  - fp16 on the whole matmul path (same PE rate as bf16, ~8x less noise)
  - PE warmup junk matmuls release the HAM clock gate before the first real
    MM; more junk matmuls in the tail keep it released through the last
    normalize so the final out-projection runs at 2.4 GHz
  - cascaded pipeline: loop w emits cx(w, t) AND sc(w+1, t) each slot, so
    the ACT engine (exp, the 285us serial floor) streams continuously across
    window seams; window w+1's exp tiles live in the xv buffer (dead after
    the v projections drain in loop 0) as a 16-region ring gated per-region
    by the v-chain reads
  - normalize: sums row staged to partition 0, reciprocal_approx_fast (DVE),
    partition_broadcast (gpsimd), ctx muls (DVE), spread as 5 deferred steps
    across the next loop
  - input DMA: few large descriptors, q-side on sync / k,v-side on gpsimd so
    the first window's kT and qT operands land concurrently
"""

import os
import sys
from collections import deque
from contextlib import ExitStack

for _p in ("/opt/trn_rl_repo", "/root/.axon_site/_ro/trn_rl_repo"):
    if os.path.isdir(_p) and _p not in sys.path:
        sys.path.insert(0, _p)
        break

import numpy as np

import concourse.bass as bass
import concourse.bacc as bacc
import concourse.mybir as mybir
import concourse.tile as tile

B, S, E, H, D = 4, 2048, 1024, 16, 64
HG = 2          # head groups (tensor-parallel factor)
DH = E // HG    # 512 dims per head group (8 heads)
HPG = H // HG   # 8 heads per group
NCORES = B * HG

EC = E // 128   # 8 contraction chunks for projections
DC = DH // 128  # 4 d-chunks (head pairs)
TC = S // 128   # 16 t chunks
ST = S // 512   # 4 s tiles
SQ = S // 512   # 4 column blocks for projections
DA = D + 1      # 65: head dim + ones column
NW = ST * DC    # 16 windows

F32 = mybir.dt.float32
MM_DT = mybir.dt.float16
SCALE = 1.0 / np.sqrt(D)

# window processing order: interleave st=0 (producer-heavy) with st=1
ORDER = [(0, 0), (1, 0), (0, 1), (1, 1), (0, 2), (1, 2), (0, 3), (1, 3),
         (2, 0), (2, 1), (2, 2), (2, 3), (3, 0), (3, 1), (3, 2), (3, 3)]


def build_nc():
    # all inputs arrive partition-packed: row p holds the concatenation over
    # e-strips of the transposed tensor's row 128e+p, so a full-tensor DMA is
    # one contiguous run per partition (~128 descriptor runs total)
    nc = bacc.Bacc()
    xqT = nc.declare_dram_parameter("xqT", [128, EC * S], MM_DT, isOutput=False)
    xkT = nc.declare_dram_parameter("xkT", [128, EC * S], MM_DT, isOutput=False)
    xvT = nc.declare_dram_parameter("xvT", [128, EC * S], MM_DT, isOutput=False)
    wqT = nc.declare_dram_parameter("wqT", [128, EC * DH], MM_DT, isOutput=False)
    wkT = nc.declare_dram_parameter("wkT", [128, EC * DH], MM_DT, isOutput=False)
    wvT = nc.declare_dram_parameter("wvT", [128, EC * DH], MM_DT, isOutput=False)
    woT = nc.declare_dram_parameter("woT", [128, DC * E], MM_DT, isOutput=False)
    out = nc.declare_dram_parameter("out", [S, E], F32, isOutput=True)

    with (
        nc.allow_low_precision(reason="fp16 matmul operands"),
        tile.TileContext(nc) as tc,
        ExitStack() as ctx,
    ):
        _emit(ctx, tc, xqT, xkT, xvT, wqT, wkT, wvT, woT, out)
    nc.compile()
    return nc


def _emit(ctx, tc, xqT, xkT, xvT, wqT, wkT, wvT, woT, out):
    nc = tc.nc

    big = ctx.enter_context(tc.tile_pool(name="big", bufs=1))
    # qT/kT/ctxT: [p, c, s] with local dim ld = 128*c + p
    # (head 2c on partitions 0-63, head 2c+1 on 64-127)
    qT_sb = big.tile([128, DC, S], MM_DT, tag="qT")
    kT_sb = big.tile([128, DC, S], MM_DT, tag="kT")
    ctxT_sb = big.tile([128, DC, S], MM_DT, tag="ctx")
    # v_aug: [t%128, t_chunk, head, 65]; col 64 is the ones column
    v_sb = big.tile([128, TC, HPG, DA], MM_DT, tag="v")
    wq_sb = big.tile([128, EC, DH], MM_DT, tag="wq")
    wk_sb = big.tile([128, EC, DH], MM_DT, tag="wk")
    wv_sb = big.tile([128, EC, DH], MM_DT, tag="wv")
    wo_sb = big.tile([128, DC, E], MM_DT, tag="wo")
    # xk is sq-major ([p, sq, e, s%512]) matching its packed DRAM layout
    xk_st = big.tile([128, SQ, EC, 512], MM_DT, tag="xk")
    # xv is stored t-chunk-major ([p, t-chunk, e*128+s]) and doubles as the
    # 16-region exp ring once the v chains drain: region t = [:, t, :] holds
    # window w+1's ex(t) during loop w, WAR-gated by exactly v(t)'s reads
    xv_st = big.tile([128, TC, 1024], MM_DT, tag="xv")
    ones = big.tile([128, 1], MM_DT, tag="ones")
    junk = big.tile([128, 512], MM_DT, tag="junk")
    nc.vector.memset(ones, 1.0)
    nc.vector.memset(junk, 0.0)
    nc.vector.tensor_copy(
        v_sb[:, :, :, D : D + 1],
        ones.to_broadcast((128, TC * HPG)).rearrange(
            "p (t h o) -> p t h o", t=TC, h=HPG
        ),
    )

    xqp = ctx.enter_context(tc.tile_pool(name="xqp", bufs=2))
    osb = ctx.enter_context(tc.tile_pool(name="osb", bufs=2))
    cxsp = ctx.enter_context(tc.tile_pool(name="cxsp", bufs=3))
    normp = ctx.enter_context(tc.tile_pool(name="normp", bufs=2))
    scps = ctx.enter_context(tc.tile_pool(name="scps", bufs=2, space="PSUM"))
    cxps = ctx.enter_context(tc.tile_pool(name="cxps", bufs=2, space="PSUM"))
    accps = ctx.enter_context(tc.tile_pool(name="accps", bufs=2, space="PSUM"))

    # ---- PE warmup: release the HAM clock gate before real matmuls --------
    jp = scps.tile([128, 1024], F32, tag="sc", name="jp")
    for _ in range(34):
        nc.tensor.matmul(jp[:, 0:512], lhsT=junk[:, 0:128], rhs=junk,
                         start=True, stop=True)

    # ---- DMA: priority-ordered; q-side on sync, k/v-side on gpsimd so the
    # first window's kT and qT operands land concurrently.  Full-tensor
    # loads are single calls (one contiguous run per partition).
    SQB = EC * 512  # 4096 packed columns per s-tile block
    nc.gpsimd.dma_start(out=wk_sb, in_=wkT[:, :].rearrange(
        "p (e d) -> p e d", e=EC))
    nc.gpsimd.dma_start(
        out=xk_st[:, 0, :, :],
        in_=xkT[:, 0:SQB].rearrange("p (e s) -> p e s", e=EC),
    )
    nc.sync.dma_start(out=wq_sb, in_=wqT[:, :].rearrange(
        "p (e d) -> p e d", e=EC))
    xq_tiles = {}
    xq_tiles[0] = xqp.tile([128, EC, 512], MM_DT, tag="xq", name="xq_st0")
    nc.sync.dma_start(
        out=xq_tiles[0], in_=xqT[:, 0:SQB].rearrange("p (e s) -> p e s", e=EC)
    )
    # v(0) gates the first exp (arena WAR), so wv + the first xv chunk come
    # right after the k-side priority loads
    nc.gpsimd.dma_start(out=wv_sb, in_=wvT[:, :].rearrange(
        "p (e d) -> p e d", e=EC))
    nc.gpsimd.dma_start(out=xv_st[:, 0:4, :], in_=xvT[:, 0 : 4 * 1024])
    xq_tiles[1] = xqp.tile([128, EC, 512], MM_DT, tag="xq", name="xq_st1")
    nc.sync.dma_start(
        out=xq_tiles[1],
        in_=xqT[:, SQB : 2 * SQB].rearrange("p (e s) -> p e s", e=EC),
    )
    nc.sync.dma_start(
        out=xk_st[:, 1:SQ, :, :],
        in_=xkT[:, SQB : SQ * SQB].rearrange(
            "p (q e s) -> p q e s", q=SQ - 1, e=EC
        ),
    )
    nc.gpsimd.dma_start(out=xv_st[:, 4:TC, :], in_=xvT[:, 4 * 1024 : EC * S])
    nc.sync.dma_start(out=wo_sb, in_=woT[:, :].rearrange(
        "p (a x) -> p a x", a=DC))

    def stage_xq(st):
        nxt = xqp.tile([128, EC, 512], MM_DT, tag="xq", name=f"xq_st{st}")
        xq_tiles[st] = nxt
        nc.sync.dma_start(
            out=nxt,
            in_=xqT[:, SQB * st : SQB * (st + 1)].rearrange(
                "p (e s) -> p e s", e=EC
            ),
        )

    # ---- producer generators (yield after each matmul) --------------------
    def kT_gen(c, sq):
        acc = accps.tile([128, 512], F32, tag="acc", name=f"kacc_{c}_{sq}")
        for e in range(EC):
            nc.tensor.matmul(
                acc,
                lhsT=wk_sb[:, e, 128 * c : 128 * (c + 1)],
                rhs=xk_st[:, sq, e, :],
                start=(e == 0),
                stop=(e == EC - 1),
            )
            yield
        nc.vector.tensor_copy(kT_sb[:, c, 512 * sq : 512 * (sq + 1)], acc)

    def v_gen(tt):
        acc = accps.tile([128, 512], F32, tag="acc", name=f"vacc_{tt}")
        for e in range(EC):
            nc.tensor.matmul(
                acc,
                lhsT=xv_st[:, tt, 128 * e : 128 * (e + 1)],
                rhs=wv_sb[:, e, :],
                start=(e == 0),
                stop=(e == EC - 1),
            )
            yield
        nc.vector.tensor_copy(
            v_sb[:, tt, :, 0:D], acc.rearrange("p (h d) -> p h d", h=HPG)
        )

    def qT_gen(st, c):
        xq = xq_tiles[st]
        acc = accps.tile([128, 512], F32, tag="acc", name=f"qacc_{st}_{c}")
        for e in range(EC):
            nc.tensor.matmul(
                acc,
                lhsT=wq_sb[:, e, 128 * c : 128 * (c + 1)],
                rhs=xq[:, e, :],
                start=(e == 0),
                stop=(e == EC - 1),
            )
            yield
        nc.vector.tensor_copy(qT_sb[:, c, 512 * st : 512 * (st + 1)], acc)

    def fp_gen(st, si):
        r0 = 512 * st + 128 * si
        for et in range(2):
            fp = accps.tile([128, 512], F32, tag="acc", name=f"fp_{r0}_{et}")
            for cc in range(DC):
                nc.tensor.matmul(
                    fp,
                    lhsT=ctxT_sb[:, cc, r0 : r0 + 128],
                    rhs=wo_sb[:, cc, 512 * et : 512 * (et + 1)],
                    start=(cc == 0),
                    stop=(cc == DC - 1),
                )
                yield
            ot = osb.tile([128, 512], F32, tag="oh", name=f"oh_{r0}_{et}")
            nc.vector.tensor_copy(ot, fp)
            nc.gpsimd.dma_start(
                out=out[r0 : r0 + 128, 512 * et : 512 * (et + 1)], in_=ot
            )

    # producer queue machinery: (key, generator) FIFO with forced drains
    producers = deque()
    done_keys = set()
    cur = [None, None]  # key, generator

    def _finish_cur():
        done_keys.add(cur[0])
        cur[0] = cur[1] = None

    def pump(n):
        emitted = 0
        while emitted < n:
            if cur[1] is None:
                if not producers:
                    return
                cur[0], cur[1] = producers.popleft()
            try:
                next(cur[1])
                emitted += 1
            except StopIteration:
                _finish_cur()

    def pump_until(key):
        while key not in done_keys:
            if cur[1] is None:
                if not producers:
                    raise RuntimeError(f"producer underflow waiting for {key}")
                cur[0], cur[1] = producers.popleft()
            try:
                while True:
                    next(cur[1])
            except StopIteration:
                _finish_cur()

    # ---- prologue: just enough for the first scores matmul ---------------
    for _ in kT_gen(0, 0):
        pass
    for _ in qT_gen(0, 0):
        pass
    done_keys.add(("kT", 0, 0))
    done_keys.add(("qT", 0, 0))

    # producer order mirrors the cascaded schedule: loop w consumes window
    # w's cx operands and emits window w+1's sc, so each window's qT/kT must
    # finish one loop earlier than in a flat schedule
    producers.append((("v", 0), v_gen(0)))
    producers.append((("v", 1), v_gen(1)))
    producers.append((("kT", 0, 1), kT_gen(0, 1)))
    producers.append((("v", 2), v_gen(2)))
    producers.append((("v", 3), v_gen(3)))
    producers.append((("kT", 0, 2), kT_gen(0, 2)))
    producers.append((("v", 4), v_gen(4)))
    producers.append((("v", 5), v_gen(5)))
    producers.append((("kT", 0, 3), kT_gen(0, 3)))
    for tt in range(6, TC):
        producers.append((("v", tt), v_gen(tt)))
    producers.append((("qT", 1, 0), qT_gen(1, 0)))
    producers.append((("qT", 0, 1), qT_gen(0, 1)))
    producers.append((("kT", 1, 0), kT_gen(1, 0)))
    for sq in range(1, SQ):
        producers.append((("kT", 1, sq), kT_gen(1, sq)))
    producers.append((("qT", 1, 1), qT_gen(1, 1)))
    producers.append((("qT", 0, 2), qT_gen(0, 2)))
    for sq in range(SQ):
        producers.append((("kT", 2, sq), kT_gen(2, sq)))
    producers.append((("qT", 1, 2), qT_gen(1, 2)))
    producers.append((("qT", 0, 3), qT_gen(0, 3)))
    for sq in range(SQ):
        producers.append((("kT", 3, sq), kT_gen(3, sq)))
    producers.append((("qT", 1, 3), qT_gen(1, 3)))
    # qT(2,*) / qT(3,*) appended at loops 5/6 once their xq slot is clear

    # ---- deferred normalize ----------------------------------------------
    norm_steps = deque()

    def run_norm_step():
        if norm_steps:
            norm_steps.popleft()()

    def queue_norm(st, c, cxs):
        s0 = 512 * st
        rec = [None, None]
        bc = [None, None]

        def recips():
            # reciprocal_approx_fast cannot read from a nonzero base
            # partition on HW (NaN) -- stage the sums row to partition 0
            for j in range(2):
                xs = normp.tile(
                    [1, 512], F32, tag="xs", name=f"xs{st}_{c}_{j}"
                )
                nc.vector.tensor_copy(xs, cxs[j][D : D + 1, :])
                rec[j] = normp.tile(
                    [1, 512], F32, tag="rec", name=f"rec{st}_{c}_{j}"
                )
                nc.vector.reciprocal_approx_fast(out=rec[j], in_=xs)

        def bcast(j):
            def fn():
                bc[j] = normp.tile(
                    [64, 512], F32, tag="bc", name=f"bc{st}_{c}_{j}"
                )
                nc.gpsimd.partition_broadcast(bc[j], rec[j])

            return fn

        def mul(j):
            def fn():
                nc.vector.tensor_mul(
                    ctxT_sb[64 * j : 64 * (j + 1), c, s0 : s0 + 512],
                    cxs[j][0:D, :],
                    bc[j],
                )
                # whole s-tile normalized -> its output projection may run
                if j == 1 and c == DC - 1:
                    for si in range(4):
                        producers.append((("fp", st, si), fp_gen(st, si)))

            return fn

        norm_steps.append(recips)
        norm_steps.append(bcast(0))
        norm_steps.append(bcast(1))
        norm_steps.append(mul(0))
        norm_steps.append(mul(1))

    # ---- main cascaded loop ----------------------------------------------
    cx_of = {}
    ex_of = {}

    def begin_window(wi):
        ex_of[wi] = {}
        cx_of[wi] = [
            cxps.tile([DA, 512], F32, tag="cx", name=f"cx{wi}_{j}")
            for j in range(2)
        ]

    def emit_sc(wi, t):
        st, c = ORDER[wi]
        if ("kT", c, t // 4) not in done_keys:
            pump_until(("kT", c, t // 4))
        s0 = 512 * st
        sc = scps.tile([128, 1024], F32, tag="sc")
        for j in range(2):
            nc.tensor.matmul(
                sc[:, 512 * j : 512 * (j + 1)],
                lhsT=kT_sb[64 * j : 64 * (j + 1), c, 128 * t : 128 * (t + 1)],
                rhs=qT_sb[64 * j : 64 * (j + 1), c, s0 : s0 + 512],
                start=True,
                stop=True,
            )
        # contiguous arena region in the xv buffer: WAR-gated by v(t)'s
        # reads (window 0) and thereafter by cx(w-1, t) -- exactly the same
        # gates its consumer cx(w, t) has, so the region is always ready
        # just in time
        ex = xv_st[:, t, :]
        nc.scalar.activation(
            out=ex, in_=sc,
            func=mybir.ActivationFunctionType.Exp, scale=float(SCALE),
        )
        ex_of[wi][t] = ex

    def emit_cx(wi, t):
        st, c = ORDER[wi]
        if ("v", t) not in done_keys:
            pump_until(("v", t))
        ex = ex_of[wi].pop(t)
        cx = cx_of[wi]
        for j in range(2):
            nc.tensor.matmul(
                cx[j],
                lhsT=v_sb[:, t, 2 * c + j, :],
                rhs=ex[:, 512 * j : 512 * (j + 1)],
                start=(t == 0),
                stop=(t == TC - 1),
            )

    def end_window(wi):
        st, c = ORDER[wi]
        cxs = []
        for j in range(2):
            t_ = cxsp.tile([DA, 512], F32, tag="cxs", name=f"cxs{wi}_{j}")
            nc.vector.tensor_copy(t_, cx_of[wi][j])
            cxs.append(t_)
        queue_norm(st, c, cxs)

    # window 0's sc/exp stream: v(t) must be emitted before ACT(0,t) so the
    # arena region's WAR gate orders the exp write after v's reads
    begin_window(0)
    for t in range(TC):
        if ("v", t) not in done_keys:
            pump_until(("v", t))
        emit_sc(0, t)
        pump(2)

    for w in range(NW):
        if w + 1 < NW:
            st1, c1 = ORDER[w + 1]
            if ("qT", st1, c1) not in done_keys:
                pump_until(("qT", st1, c1))
            begin_window(w + 1)
        for t in range(TC):
            emit_cx(w, t)
            if w + 1 < NW:
                emit_sc(w + 1, t)
            pump(2)
            if t >= 2:
                run_norm_step()
        end_window(w)
        if w == 5:
            # qT(0,3) fully emitted (pump_until at loop-5 start), so xq st2
            # may take st0's slot now
            stage_xq(2)
            for c2 in range(DC):
                producers.append((("qT", 2, c2), qT_gen(2, c2)))
        if w == 6:
            stage_xq(3)
            for c2 in range(DC):
                producers.append((("qT", 3, c2), qT_gen(3, c2)))

    # ---- epilogue ---------------------------------------------------------
    # junk matmuls bridge the PE-idle gap while the last normalize chain
    # runs, so the HAM clock gate stays open for the final out-projection
    jp2 = scps.tile([128, 1024], F32, tag="sc", name="jp2")
    for _ in range(14):
        nc.tensor.matmul(jp2[:, 0:512], lhsT=junk[:, 0:128], rhs=junk,
                         start=True, stop=True)
    while norm_steps:
        run_norm_step()
    pump(10**9)


_BUILT = {}


def _get_nc():
    if "nc" not in _BUILT:
        _BUILT["nc"] = build_nc()
    return _BUILT["nc"]


def make_in_maps(query, key, value, Wq, Wk, Wv, Wo):
    ndt = mybir.dt.np(MM_DT)
    query = np.asarray(query, np.float32).astype(ndt)
    key = np.asarray(key, np.float32).astype(ndt)
    value = np.asarray(value, np.float32).astype(ndt)
    Wq = np.asarray(Wq, np.float32).astype(ndt)
    Wk = np.asarray(Wk, np.float32).astype(ndt)
    Wv = np.asarray(Wv, np.float32).astype(ndt)
    Wo = np.asarray(Wo, np.float32).astype(ndt)

    def pack(xT, width):
        # [rows, width] -> [128, (rows/128)*width]: row p holds the
        # concatenation over e of xT[128e+p, :]
        e = xT.shape[0] // 128
        return np.ascontiguousarray(
            xT.reshape(e, 128, width).transpose(1, 0, 2).reshape(128, e * width)
        )

    def pack_sq(xT):
        # [E, S] -> [128, EC*S] with row p = [sq][e][s%512]
        return np.ascontiguousarray(
            xT.reshape(EC, 128, SQ, 512)
            .transpose(1, 2, 0, 3)
            .reshape(128, EC * S)
        )

    xqT = [pack_sq(query[b].T) for b in range(B)]
    xkT = [pack_sq(key[b].T) for b in range(B)]
    # xv t-chunk-major: row p = [t-chunk][e][s%128]
    xvT = [
        np.ascontiguousarray(
            value[b].T.reshape(EC, 128, TC, 128)
            .transpose(1, 2, 0, 3)
            .reshape(128, EC * S)
        )
        for b in range(B)
    ]
    wqT = [pack(Wq[DH * g : DH * (g + 1), :].T, DH) for g in range(HG)]
    wkT = [pack(Wk[DH * g : DH * (g + 1), :].T, DH) for g in range(HG)]
    wvT = [pack(Wv[DH * g : DH * (g + 1), :].T, DH) for g in range(HG)]
    woT = [pack(Wo[:, DH * g : DH * (g + 1)].T, E) for g in range(HG)]

    in_maps = []
    for core in range(NCORES):
        b, g = core // HG, core % HG
        in_maps.append(
            {
                "xqT": xqT[b],
                "xkT": xkT[b],
                "xvT": xvT[b],
                "wqT": wqT[g],
                "wkT": wkT[g],
                "wvT": wvT[g],
                "woT": woT[g],
            }
        )
    return in_maps


def assemble(core_outs):
    out = np.empty((B, S, E), np.float32)
    for b in range(B):
        out[b] = core_outs[HG * b]
        for g in range(1, HG):
            out[b] += core_outs[HG * b + g]
    return out


def kernel(query, key, value, Wq, Wk, Wv, Wo):
    from concourse.bass_utils import run_bass_kernel_spmd

    nc = _get_nc()
    in_maps = make_in_maps(query, key, value, Wq, Wk, Wv, Wo)
    res = run_bass_kernel_spmd(nc, in_maps, list(range(NCORES)))
    return assemble([r["out"] for r in res.results])


# revision 24
# speedup vs baseline: 1.1746x; 1.1746x over previous
"""Multi-head attention TRN2 kernel (v5, fp16, cascaded window pipeline).

Sharding: 8 cores = 4 batches x 2 head-groups (Megatron tensor parallel over
the 16 heads: Wq/Wk/Wv column-sharded, Wo row-sharded; partial outputs summed
per batch on the host).

Per-core schedule (batch b, head-group g -> 8 local heads, 4 head-pairs c,
window = (s-tile st, head-pair c), 16 windows):
  - fp16 on the whole matmul path (same PE rate as bf16, ~8x less noise)
  - PE warmup junk matmuls release the HAM clock gate before the first real
    MM; more junk matmuls in the tail keep it released through the last
    normalize so the final out-projection runs at 2.4 GHz
  - cascaded pipeline: loop w emits cx(w, t) AND sc(w+1, t) each slot, so
    the ACT engine (exp, the 285us serial floor) streams continuously across
    window seams; window w+1's exp tiles live in the xv buffer (dead after
    the v projections drain in loop 0) as a 16-region ring gated per-region
    by the v-chain reads
  - normalize: sums row staged to partition 0, reciprocal_approx_fast (DVE),
    partition_broadcast (gpsimd), ctx muls (DVE), spread as 5 deferred steps
    across the next loop
  - input DMA: few large descriptors, q-side on sync / k,v-side on gpsimd so
    the first window's kT and qT operands land concurrently
"""

import os
import sys
from collections import deque
from contextlib import ExitStack

for _p in ("/opt/trn_rl_repo", "/root/.axon_site/_ro/trn_rl_repo"):
    if os.path.isdir(_p) and _p not in sys.path:
        sys.path.insert(0, _p)
        break

import numpy as np

import concourse.bass as bass
import concourse.bacc as bacc
import concourse.mybir as mybir
import concourse.tile as tile

B, S, E, H, D = 4, 2048, 1024, 16, 64
HG = 2          # head groups (tensor-parallel factor)
DH = E // HG    # 512 dims per head group (8 heads)
HPG = H // HG   # 8 heads per group
NCORES = B * HG

EC = E // 128   # 8 contraction chunks for projections
DC = DH // 128  # 4 d-chunks (head pairs)
TC = S // 128   # 16 t chunks
ST = S // 512   # 4 s tiles
SQ = S // 512   # 4 column blocks for projections
DA = D + 1      # 65: head dim + ones column
NW = ST * DC    # 16 windows

F32 = mybir.dt.float32
MM_DT = mybir.dt.float16
SCALE = 1.0 / np.sqrt(D)

# window processing order: interleave st=0 (producer-heavy) with st=1
ORDER = [(0, 0), (1, 0), (0, 1), (1, 1), (0, 2), (1, 2), (0, 3), (1, 3),
         (2, 0), (2, 1), (2, 2), (2, 3), (3, 0), (3, 1), (3, 2), (3, 3)]


def build_nc():
    # all inputs arrive partition-packed: row p holds the concatenation over
    # e-strips of the transposed tensor's row 128e+p, so a full-tensor DMA is
    # one contiguous run per partition (~128 descriptor runs total)
    nc = bacc.Bacc()
    xqT = nc.declare_dram_parameter("xqT", [128, EC * S], MM_DT, isOutput=False)
    xkT = nc.declare_dram_parameter("xkT", [128, EC * S], MM_DT, isOutput=False)
    xvT = nc.declare_dram_parameter("xvT", [128, EC * S], MM_DT, isOutput=False)
    wqT = nc.declare_dram_parameter("wqT", [128, EC * DH], MM_DT, isOutput=False)
    wkT = nc.declare_dram_parameter("wkT", [128, EC * DH], MM_DT, isOutput=False)
    wvT = nc.declare_dram_parameter("wvT", [128, EC * DH], MM_DT, isOutput=False)
    woT = nc.declare_dram_parameter("woT", [128, DC * E], MM_DT, isOutput=False)
    out = nc.declare_dram_parameter("out", [S, E], F32, isOutput=True)

    with (
        nc.allow_low_precision(reason="fp16 matmul operands"),
        tile.TileContext(nc) as tc,
        ExitStack() as ctx,
    ):
        _emit(ctx, tc, xqT, xkT, xvT, wqT, wkT, wvT, woT, out)
    nc.compile()
    return nc


def _emit(ctx, tc, xqT, xkT, xvT, wqT, wkT, wvT, woT, out):
    nc = tc.nc

    big = ctx.enter_context(tc.tile_pool(name="big", bufs=1))
    # qT/kT/ctxT: [p, c, s] with local dim ld = 128*c + p
    # (head 2c on partitions 0-63, head 2c+1 on 64-127)
    qT_sb = big.tile([128, DC, S], MM_DT, tag="qT")
    kT_sb = big.tile([128, DC, S], MM_DT, tag="kT")
    ctxT_sb = big.tile([128, DC, S], MM_DT, tag="ctx")
    # v_aug: [t%128, t_chunk, head, 65]; col 64 is the ones column
    v_sb = big.tile([128, TC, HPG, DA], MM_DT, tag="v")
    wq_sb = big.tile([128, EC, DH], MM_DT, tag="wq")
    wk_sb = big.tile([128, EC, DH], MM_DT, tag="wk")
    wv_sb = big.tile([128, EC, DH], MM_DT, tag="wv")
    wo_sb = big.tile([128, DC, E], MM_DT, tag="wo")
    # xk is sq-major ([p, sq, e, s%512]) matching its packed DRAM layout
    xk_st = big.tile([128, SQ, EC, 512], MM_DT, tag="xk")
    # xv is stored t-chunk-major ([p, t-chunk, e*128+s]) and doubles as the
    # 16-region exp ring once the v chains drain: region t = [:, t, :] holds
    # window w+1's ex(t) during loop w, WAR-gated by exactly v(t)'s reads
    xv_st = big.tile([128, TC, 1024], MM_DT, tag="xv")
    ones = big.tile([128, 1], MM_DT, tag="ones")
    junk = big.tile([128, 512], MM_DT, tag="junk")
    nc.vector.memset(ones, 1.0)
    nc.vector.memset(junk, 0.0)
    nc.vector.tensor_copy(
        v_sb[:, :, :, D : D + 1],
        ones.to_broadcast((128, TC * HPG)).rearrange(
            "p (t h o) -> p t h o", t=TC, h=HPG
        ),
    )

    xqp = ctx.enter_context(tc.tile_pool(name="xqp", bufs=2))
    expool = ctx.enter_context(tc.tile_pool(name="ex", bufs=4))
    osb = ctx.enter_context(tc.tile_pool(name="osb", bufs=2))
    cxsp = ctx.enter_context(tc.tile_pool(name="cxsp", bufs=3))
    normp = ctx.enter_context(tc.tile_pool(name="normp", bufs=2))
    scps = ctx.enter_context(tc.tile_pool(name="scps", bufs=2, space="PSUM"))
    cxps = ctx.enter_context(tc.tile_pool(name="cxps", bufs=2, space="PSUM"))
    accps = ctx.enter_context(tc.tile_pool(name="accps", bufs=2, space="PSUM"))

    # ---- PE warmup: release the HAM clock gate before real matmuls --------
    jp = scps.tile([128, 1024], F32, tag="sc", name="jp")
    for _ in range(34):
        nc.tensor.matmul(jp[:, 0:512], lhsT=junk[:, 0:128], rhs=junk,
                         start=True, stop=True)

    # ---- DMA: priority-ordered; q-side on sync, k/v-side on gpsimd so the
    # first window's kT and qT operands land concurrently.  Full-tensor
    # loads are single calls (one contiguous run per partition).
    SQB = EC * 512  # 4096 packed columns per s-tile block
    nc.gpsimd.dma_start(out=wk_sb, in_=wkT[:, :].rearrange(
        "p (e d) -> p e d", e=EC))
    nc.gpsimd.dma_start(
        out=xk_st[:, 0, :, :],
        in_=xkT[:, 0:SQB].rearrange("p (e s) -> p e s", e=EC),
    )
    nc.sync.dma_start(out=wq_sb, in_=wqT[:, :].rearrange(
        "p (e d) -> p e d", e=EC))
    xq_tiles = {}
    xq_tiles[0] = xqp.tile([128, EC, 512], MM_DT, tag="xq", name="xq_st0")
    nc.sync.dma_start(
        out=xq_tiles[0], in_=xqT[:, 0:SQB].rearrange("p (e s) -> p e s", e=EC)
    )
    # v(0) gates the first exp (arena WAR), so wv + the first xv chunk come
    # right after the k-side priority loads
    nc.gpsimd.dma_start(out=wv_sb, in_=wvT[:, :].rearrange(
        "p (e d) -> p e d", e=EC))
    nc.gpsimd.dma_start(out=xv_st[:, 0:4, :], in_=xvT[:, 0 : 4 * 1024])
    xq_tiles[1] = xqp.tile([128, EC, 512], MM_DT, tag="xq", name="xq_st1")
    nc.sync.dma_start(
        out=xq_tiles[1],
        in_=xqT[:, SQB : 2 * SQB].rearrange("p (e s) -> p e s", e=EC),
    )
    nc.sync.dma_start(
        out=xk_st[:, 1, :, :],
        in_=xkT[:, SQB : 2 * SQB].rearrange("p (e s) -> p e s", e=EC),
    )
    nc.sync.dma_start(
        out=xk_st[:, 2:SQ, :, :],
        in_=xkT[:, 2 * SQB : SQ * SQB].rearrange(
            "p (q e s) -> p q e s", q=SQ - 2, e=EC
        ),
    )
    nc.gpsimd.dma_start(out=xv_st[:, 4:TC, :], in_=xvT[:, 4 * 1024 : EC * S])
    nc.sync.dma_start(out=wo_sb, in_=woT[:, :].rearrange(
        "p (a x) -> p a x", a=DC))

    def stage_xq(st):
        nxt = xqp.tile([128, EC, 512], MM_DT, tag="xq", name=f"xq_st{st}")
        xq_tiles[st] = nxt
        nc.sync.dma_start(
            out=nxt,
            in_=xqT[:, SQB * st : SQB * (st + 1)].rearrange(
                "p (e s) -> p e s", e=EC
            ),
        )

    # ---- producer generators (yield after each matmul) --------------------
    def kT_gen(c, sq):
        acc = accps.tile([128, 512], F32, tag="acc", name=f"kacc_{c}_{sq}")
        for e in range(EC):
            nc.tensor.matmul(
                acc,
                lhsT=wk_sb[:, e, 128 * c : 128 * (c + 1)],
                rhs=xk_st[:, sq, e, :],
                start=(e == 0),
                stop=(e == EC - 1),
            )
            yield
        nc.vector.tensor_copy(kT_sb[:, c, 512 * sq : 512 * (sq + 1)], acc)

    def v_gen(tt):
        acc = accps.tile([128, 512], F32, tag="acc", name=f"vacc_{tt}")
        for e in range(EC):
            nc.tensor.matmul(
                acc,
                lhsT=xv_st[:, tt, 128 * e : 128 * (e + 1)],
                rhs=wv_sb[:, e, :],
                start=(e == 0),
                stop=(e == EC - 1),
            )
            yield
        nc.vector.tensor_copy(
            v_sb[:, tt, :, 0:D], acc.rearrange("p (h d) -> p h d", h=HPG)
        )

    def qT_gen(st, c):
        xq = xq_tiles[st]
        acc = accps.tile([128, 512], F32, tag="acc", name=f"qacc_{st}_{c}")
        for e in range(EC):
            nc.tensor.matmul(
                acc,
                lhsT=wq_sb[:, e, 128 * c : 128 * (c + 1)],
                rhs=xq[:, e, :],
                start=(e == 0),
                stop=(e == EC - 1),
            )
            yield
        nc.vector.tensor_copy(qT_sb[:, c, 512 * st : 512 * (st + 1)], acc)

    def fp_gen(st, si):
        r0 = 512 * st + 128 * si
        for et in range(2):
            fp = accps.tile([128, 512], F32, tag="acc", name=f"fp_{r0}_{et}")
            for cc in range(DC):
                nc.tensor.matmul(
                    fp,
                    lhsT=ctxT_sb[:, cc, r0 : r0 + 128],
                    rhs=wo_sb[:, cc, 512 * et : 512 * (et + 1)],
                    start=(cc == 0),
                    stop=(cc == DC - 1),
                )
                yield
            ot = osb.tile([128, 512], F32, tag="oh", name=f"oh_{r0}_{et}")
            nc.vector.tensor_copy(ot, fp)
            nc.gpsimd.dma_start(
                out=out[r0 : r0 + 128, 512 * et : 512 * (et + 1)], in_=ot
            )

    # producer queue machinery: (key, generator) FIFO with forced drains
    producers = deque()
    done_keys = set()
    cur = [None, None]  # key, generator

    def _finish_cur():
        done_keys.add(cur[0])
        cur[0] = cur[1] = None

    def pump(n):
        emitted = 0
        while emitted < n:
            if cur[1] is None:
                if not producers:
                    return
                cur[0], cur[1] = producers.popleft()
            try:
                next(cur[1])
                emitted += 1
            except StopIteration:
                _finish_cur()

    def pump_until(key):
        while key not in done_keys:
            if cur[1] is None:
                if not producers:
                    raise RuntimeError(f"producer underflow waiting for {key}")
                cur[0], cur[1] = producers.popleft()
            try:
                while True:
                    next(cur[1])
            except StopIteration:
                _finish_cur()

    # ---- prologue: just enough for the first scores matmul ---------------
    for _ in kT_gen(0, 0):
        pass
    for _ in qT_gen(0, 0):
        pass
    done_keys.add(("kT", 0, 0))
    done_keys.add(("qT", 0, 0))

    # producer order mirrors the cascaded schedule: loop w consumes window
    # w's cx operands and emits window w+1's sc, so each window's qT/kT must
    # finish one loop earlier than in a flat schedule
    for tt in range(4):
        producers.append((("v", tt), v_gen(tt)))
    producers.append((("kT", 0, 1), kT_gen(0, 1)))
    producers.append((("v", 4), v_gen(4)))
    producers.append((("v", 5), v_gen(5)))
    producers.append((("kT", 0, 2), kT_gen(0, 2)))
    producers.append((("v", 6), v_gen(6)))
    producers.append((("v", 7), v_gen(7)))
    producers.append((("kT", 0, 3), kT_gen(0, 3)))
    for tt in range(8, TC):
        producers.append((("v", tt), v_gen(tt)))
    producers.append((("qT", 1, 0), qT_gen(1, 0)))
    producers.append((("qT", 0, 1), qT_gen(0, 1)))
    producers.append((("kT", 1, 0), kT_gen(1, 0)))
    for sq in range(1, SQ):
        producers.append((("kT", 1, sq), kT_gen(1, sq)))
    producers.append((("qT", 1, 1), qT_gen(1, 1)))
    producers.append((("qT", 0, 2), qT_gen(0, 2)))
    for sq in range(SQ):
        producers.append((("kT", 2, sq), kT_gen(2, sq)))
    producers.append((("qT", 1, 2), qT_gen(1, 2)))
    producers.append((("qT", 0, 3), qT_gen(0, 3)))
    for sq in range(SQ):
        producers.append((("kT", 3, sq), kT_gen(3, sq)))
    producers.append((("qT", 1, 3), qT_gen(1, 3)))
    # qT(2,*) / qT(3,*) appended at loops 5/6 once their xq slot is clear

    # ---- deferred normalize ----------------------------------------------
    norm_steps = deque()

    def run_norm_step():
        if norm_steps:
            norm_steps.popleft()()

    def queue_norm(st, c, cxs):
        s0 = 512 * st
        rec = [None, None]
        bc = [None, None]

        def recips():
            # reciprocal_approx_fast cannot read from a nonzero base
            # partition on HW (NaN) -- stage the sums row to partition 0
            for j in range(2):
                xs = normp.tile(
                    [1, 512], F32, tag="xs", name=f"xs{st}_{c}_{j}"
                )
                nc.vector.tensor_copy(xs, cxs[j][D : D + 1, :])
                rec[j] = normp.tile(
                    [1, 512], F32, tag="rec", name=f"rec{st}_{c}_{j}"
                )
                nc.vector.reciprocal_approx_fast(out=rec[j], in_=xs)

        def bcast(j):
            def fn():
                bc[j] = normp.tile(
                    [64, 512], F32, tag="bc", name=f"bc{st}_{c}_{j}"
                )
                nc.gpsimd.partition_broadcast(bc[j], rec[j])

            return fn

        def mul(j):
            def fn():
                nc.vector.tensor_mul(
                    ctxT_sb[64 * j : 64 * (j + 1), c, s0 : s0 + 512],
                    cxs[j][0:D, :],
                    bc[j],
                )
                # whole s-tile normalized -> its output projection may run
                if j == 1 and c == DC - 1:
                    for si in range(4):
                        producers.append((("fp", st, si), fp_gen(st, si)))

            return fn

        norm_steps.append(recips)
        norm_steps.append(bcast(0))
        norm_steps.append(bcast(1))
        norm_steps.append(mul(0))
        norm_steps.append(mul(1))

    # ---- main cascaded loop ----------------------------------------------
    cx_of = {}
    ex_of = {}

    def begin_window(wi):
        ex_of[wi] = {}
        cx_of[wi] = [
            cxps.tile([DA, 512], F32, tag="cx", name=f"cx{wi}_{j}")
            for j in range(2)
        ]

    def emit_sc(wi, t):
        st, c = ORDER[wi]
        if ("kT", c, t // 4) not in done_keys:
            pump_until(("kT", c, t // 4))
        s0 = 512 * st
        sc = scps.tile([128, 1024], F32, tag="sc")
        for j in range(2):
            nc.tensor.matmul(
                sc[:, 512 * j : 512 * (j + 1)],
                lhsT=kT_sb[64 * j : 64 * (j + 1), c, 128 * t : 128 * (t + 1)],
                rhs=qT_sb[64 * j : 64 * (j + 1), c, s0 : s0 + 512],
                start=True,
                stop=True,
            )
        # contiguous arena region in the xv buffer: WAR-gated by v(t)'s
        # reads (window 0) and thereafter by cx(w-1, t) -- exactly the same
        # gates its consumer cx(w, t) has, so the region is always ready
        # just in time.  Window 0's first tiles use a small SBUF pool so the
        # exp stream starts before any v data has arrived.
        if wi == 0 and t < 4:
            ex = expool.tile([128, 1024], MM_DT, tag="ex")
        else:
            ex = xv_st[:, t, :]
        nc.scalar.activation(
            out=ex, in_=sc,
            func=mybir.ActivationFunctionType.Exp, scale=float(SCALE),
        )
        ex_of[wi][t] = ex

    def emit_cx(wi, t):
        st, c = ORDER[wi]
        if ("v", t) not in done_keys:
            pump_until(("v", t))
        ex = ex_of[wi].pop(t)
        cx = cx_of[wi]
        for j in range(2):
            nc.tensor.matmul(
                cx[j],
                lhsT=v_sb[:, t, 2 * c + j, :],
                rhs=ex[:, 512 * j : 512 * (j + 1)],
                start=(t == 0),
                stop=(t == TC - 1),
            )

    def end_window(wi):
        st, c = ORDER[wi]
        cxs = []
        for j in range(2):
            t_ = cxsp.tile([DA, 512], F32, tag="cxs", name=f"cxs{wi}_{j}")
            nc.vector.tensor_copy(t_, cx_of[wi][j])
            cxs.append(t_)
        queue_norm(st, c, cxs)

    # window 0's sc/exp stream: v(t) must be emitted before ACT(0,t) so the
    # arena region's WAR gate orders the exp write after v's reads
    begin_window(0)
    for t in range(TC):
        if t >= 4 and ("v", t) not in done_keys:
            pump_until(("v", t))
        emit_sc(0, t)
        pump(2)

    for w in range(NW):
        if w + 1 < NW:
            st1, c1 = ORDER[w + 1]
            if ("qT", st1, c1) not in done_keys:
                pump_until(("qT", st1, c1))
            begin_window(w + 1)
        for t in range(TC):
            emit_cx(w, t)
            if w + 1 < NW:
                emit_sc(w + 1, t)
            pump(2)
            if t >= 2:
                run_norm_step()
        end_window(w)
        if w == 5:
            # qT(0,3) fully emitted (pump_until at loop-5 start), so xq st2
            # may take st0's slot now
            stage_xq(2)
            for c2 in range(DC):
                producers.append((("qT", 2, c2), qT_gen(2, c2)))
        if w == 6:
            stage_xq(3)
            for c2 in range(DC):
                producers.append((("qT", 3, c2), qT_gen(3, c2)))

    # ---- epilogue ---------------------------------------------------------
    # junk matmuls bridge the PE-idle gap while the last normalize chain
    # runs, so the HAM clock gate stays open for the final out-projection
    jp2 = scps.tile([128, 1024], F32, tag="sc", name="jp2")
    for _ in range(14):
        nc.tensor.matmul(jp2[:, 0:512], lhsT=junk[:, 0:128], rhs=junk,
                         start=True, stop=True)
    while norm_steps:
        run_norm_step()
    pump(10**9)


_BUILT = {}


def _get_nc():
    if "nc" not in _BUILT:
        _BUILT["nc"] = build_nc()
    return _BUILT["nc"]


def make_in_maps(query, key, value, Wq, Wk, Wv, Wo):
    ndt = mybir.dt.np(MM_DT)
    query = np.asarray(query, np.float32).astype(ndt)
    key = np.asarray(key, np.float32).astype(ndt)
    value = np.asarray(value, np.float32).astype(ndt)
    Wq = np.asarray(Wq, np.float32).astype(ndt)
    Wk = np.asarray(Wk, np.float32).astype(ndt)
    Wv = np.asarray(Wv, np.float32).astype(ndt)
    Wo = np.asarray(Wo, np.float32).astype(ndt)

    def pack(xT, width):
        # [rows, width] -> [128, (rows/128)*width]: row p holds the
        # concatenation over e of xT[128e+p, :]
        e = xT.shape[0] // 128
        return np.ascontiguousarray(
            xT.reshape(e, 128, width).transpose(1, 0, 2).reshape(128, e * width)
        )

    def pack_sq(xT):
        # [E, S] -> [128, EC*S] with row p = [sq][e][s%512]
        return np.ascontiguousarray(
            xT.reshape(EC, 128, SQ, 512)
            .transpose(1, 2, 0, 3)
            .reshape(128, EC * S)
        )

    xqT = [pack_sq(query[b].T) for b in range(B)]
    xkT = [pack_sq(key[b].T) for b in range(B)]
    # xv t-chunk-major: row p = [t-chunk][e][s%128]
    xvT = [
        np.ascontiguousarray(
            value[b].T.reshape(EC, 128, TC, 128)
            .transpose(1, 2, 0, 3)
            .reshape(128, EC * S)
        )
        for b in range(B)
    ]
    wqT = [pack(Wq[DH * g : DH * (g + 1), :].T, DH) for g in range(HG)]
    wkT = [pack(Wk[DH * g : DH * (g + 1), :].T, DH) for g in range(HG)]
    wvT = [pack(Wv[DH * g : DH * (g + 1), :].T, DH) for g in range(HG)]
    woT = [pack(Wo[:, DH * g : DH * (g + 1)].T, E) for g in range(HG)]

    in_maps = []
    for core in range(NCORES):
        b, g = core // HG, core % HG
        in_maps.append(
            {
                "xqT": xqT[b],
                "xkT": xkT[b],
                "xvT": xvT[b],
                "wqT": wqT[g],
                "wkT": wkT[g],
                "wvT": wvT[g],
                "woT": woT[g],
            }
        )
    return in_maps


def assemble(core_outs):
    out = np.empty((B, S, E), np.float32)
    for b in range(B):
        out[b] = core_outs[HG * b]
        for g in range(1, HG):
            out[b] += core_outs[HG * b + g]
    return out


def kernel(query, key, value, Wq, Wk, Wv, Wo):
    from concourse.bass_utils import run_bass_kernel_spmd

    nc = _get_nc()
    in_maps = make_in_maps(query, key, value, Wq, Wk, Wv, Wo)
    res = run_bass_kernel_spmd(nc, in_maps, list(range(NCORES)))
    return assemble([r["out"] for r in res.results])
